# revision 61
# baseline (speedup 1.0000x reference)
"""Trainium2 Bass kernel for MemorySpatialAttention.

Math (per batch b):
  f = LeakyReLU_0.1(BN(conv(x)))  with conv = full-length dot -> x[N,L] @ W[L,H]
  sim = f_in @ f_mem^T  banded to |i-j| <= 8 (17 neighbors, clamped at edges)
  attn = softmax_band(sim);  out = 0.5*x + 0.5*(attn @ mem)

Sharding: data-parallel over batch B=8 -> one batch per NeuronCore, no
collectives.

Per-core structure: features in [H=128, N] layout (fp16 operands -> 1-pass
matmuls). Queries tiled 112 rows at a time with a uniform 128-wide key
window; tiles processed in groups of 4 sharing one PSUM bank so five groups
pipeline across engines. Feature activations are split into 512-wide chunks
interleaved with the group emission so group 0's QK only waits for the
first chunk pair (x/mem ship as fp16 in three DMAs, the first carrying just
the weights + first chunk so it lands early). The band mask is 3 distinct
[112,128] bf16 tiles (first/mid/last) broadcast with stride-0 APs. The
softmax chain is spread across engines: mask-add + row-max + all other
PSUM-reading ops on Vector, the max-subtract on GpSimd, exp + the psT->SBUF
copy on Scalar. The softmax denominator rides the attn@mem matmul as an
appended ones-column of memNB; the epilogue is a fused per-tile
(psC * 1/denom) + 0.5x scalar_tensor_tensor on Vector writing fp16.
Output (fp16) DMA is issued per-group on alternating Sync/Scalar queues
so transfers overlap the remaining compute and the issues don't serialize.
"""
import sys
sys.path.insert(0, '/opt/trn_rl_repo')

import numpy as np

B, N, C, L, H = 8, 2048, 1, 56, 128
NB, HALF = 17, 8
RT = 112              # query rows per tile
WIN = 128             # key window per tile
T = (N + RT - 1) // RT  # 19 tiles (last partial: 32 rows)
GRP = 4
NG = (T + GRP - 1) // GRP  # 5 groups (4,4,4,4,3 tiles)
NPAD = RT * T         # 2128
RATE = 0.5
BN_EPS = 1e-5
NEG_SLOPE = 0.1
FI_PAD = NPAD         # fiT cols (2128)
FM_PAD = HALF + N + (RT * (T - 1) + WIN - N)  # 8 + 2048 + 88 = 2144

_cache = {}


def _build_program():
    import concourse.bass as bass
    import concourse.bacc as bacc
    import concourse.tile as tile
    from concourse import mybir

    F32 = mybir.dt.float32
    F16 = mybir.dt.float16
    BF16 = mybir.dt.bfloat16
    AF = mybir.ActivationFunctionType
    AX = mybir.AxisListType
    from concourse.alu_op_type import AluOpType as ALU

    def bcast(ap_slice, n):
        return bass.AP(tensor=ap_slice.tensor, offset=ap_slice.offset,
                       ap=[*ap_slice.ap, [0, n]])

    def bcast_mid(ap_slice, k):
        # [112, 1, 128] slice -> [112, k, 128] with stride-0 middle dim
        ap = [list(d) for d in ap_slice.ap]
        assert ap[1][1] == 1, ap
        ap[1] = [0, k]
        return bass.AP(tensor=ap_slice.tensor, offset=ap_slice.offset, ap=ap)

    nc = bacc.Bacc("TRN2", target_bir_lowering=False, debug=False)

    xmA0 = nc.dram_tensor("xmA0", [L, H + 1024], F16, kind="ExternalInput")
    xmA1 = nc.dram_tensor("xmA1", [L, 1024], F16, kind="ExternalInput")
    xmB = nc.dram_tensor("xmB", [L, N], F16, kind="ExternalInput")
    scb = nc.dram_tensor("scb", [H, 2], F32, kind="ExternalInput")
    idt = nc.dram_tensor("idt", [RT, RT], BF16, kind="ExternalInput")
    maskD = nc.dram_tensor("maskD", [RT, 3 * WIN], BF16, kind="ExternalInput")
    memNB = nc.dram_tensor("memNB", [128, T * (L + 1)], BF16, kind="ExternalInput")
    xhp = nc.dram_tensor("xhp", [RT, T * L], F16, kind="ExternalInput")
    out = nc.dram_tensor("out", [RT, T * L], F16, kind="ExternalOutput")

    with tile.TileContext(nc) as tc:
        with tc.tile_pool(name="consts", bufs=1) as consts, \
             tc.tile_pool(name="work", bufs=4) as work, \
             tc.tile_pool(name="pbig", bufs=4, space="PSUM") as pbig, \
             tc.tile_pool(name="ptc", bufs=3, space="PSUM") as ptc:

            xmA0_s = consts.tile([L, H + 1024], F16)
            xmA1_s = consts.tile([L, 1024], F16)
            xmB_s = consts.tile([L, N], F16)
            wT_s = xmA0_s[:, 0:H]
            scb_s = consts.tile([H, 2], F32)
            maskD_s = consts.tile([RT, 3, WIN], BF16)
            memNB_s = consts.tile([128, T, L + 1], BF16)
            xh_s = consts.tile([RT, T, L], F16)
            ident = consts.tile([RT, RT], BF16)
            fiT = consts.tile([H, FI_PAD], F16)
            fmT = consts.tile([H, FM_PAD], F16)
            simS = consts.tile([RT, T, WIN], F32)
            simH = consts.tile([RT, T, WIN], BF16)
            simB = consts.tile([RT, T, WIN], BF16)
            EB = consts.tile([RT, T, WIN], BF16)
            negmax = consts.tile([RT, T], F32)
            rinv = consts.tile([RT, T], F32)
            outn = consts.tile([RT, T, L], F16)

            # input DMAs, first-needed first, spread over the 3 DMA queues;
            # xmA0 (weights + first x/mem chunk) goes alone first so the
            # first feature matmul isn't stuck behind the whole input set
            nc.sync.dma_start(out=xmA0_s, in_=xmA0.ap())
            nc.scalar.dma_start(out=scb_s, in_=scb.ap())
            nc.scalar.dma_start(out=xmB_s, in_=xmB.ap())
            nc.gpsimd.dma_start(out=xmA1_s, in_=xmA1.ap())
            nc.gpsimd.dma_start(out=maskD_s, in_=maskD.ap().rearrange(
                "p (t w) -> p t w", w=WIN))
            nc.gpsimd.dma_start(out=ident, in_=idt.ap())
            nc.gpsimd.dma_start(out=memNB_s, in_=memNB.ap().rearrange(
                "p (t d) -> p t d", d=L + 1))
            nc.gpsimd.dma_start(out=xh_s, in_=xhp.ap().rearrange(
                "p (t d) -> p t d", d=L))
            nc.gpsimd.memset(fmT[:, 0:HALF], 0.0)
            nc.gpsimd.memset(fmT[:, HALF + N:FM_PAD], 0.0)
            nc.gpsimd.memset(fiT[:, N:FI_PAD], 0.0)

            # ---- features in 512-col chunks: f^T = PRelu(BN(W^T @ x^T)) ----
            CH = 512

            def feat_chunk(q):  # q = 0..3 -> cols [512q, 512q+512) of x/mem
                for dst, off, m in ((fiT, 0, 0), (fmT, HALF, 1)):
                    psF = pbig.tile([128, CH], F32, tag="pbig", name="psF")
                    if q == 0:
                        src = xmA0_s[:, H + CH * m:H + CH * (m + 1)]
                    elif q == 1:
                        src = xmA1_s[:, CH * m:CH * (m + 1)]
                    else:
                        src = xmB_s[:, 1024 * m + CH * (q - 2):1024 * m + CH * (q - 1)]
                    nc.tensor.matmul(psF, lhsT=wT_s, rhs=src, start=True, stop=True)
                    nc.scalar.activation(dst[:, off + CH * q:off + CH * (q + 1)],
                                         psF, AF.Prelu, bias=scb_s[:, 1:2],
                                         scale=scb_s[:, 0:1], alpha=NEG_SLOPE)

            # group g needs feature cols up to ~448(g+1)+8: chunk (0,1,2,3)
            # ready before groups (0, 1, 2, 3) respectively.
            feat_chunk(0)
            feat_chunk(1)

            # ---- banded attention in groups of GRP tiles ----
            for g in range(NG):
                if g >= 1 and g + 1 < 4:
                    feat_chunk(g + 1)

                tiles = list(range(g * GRP, min((g + 1) * GRP, T)))
                K = len(tiles)
                t0 = tiles[0]

                psA = pbig.tile([RT, GRP, WIN], F32, tag="pbig", name="psA")
                for k, t in enumerate(tiles):
                    nc.tensor.matmul(psA[:, k, :], lhsT=fiT[:, RT * t:RT * (t + 1)],
                                     rhs=fmT[:, RT * t:RT * t + WIN],
                                     start=True, stop=True)

                # sS = psA + mask; one add per run of equal mask tile
                sS = simS[:, t0:t0 + K, :]
                runs = []  # (k_start, count, mi)
                for k, t in enumerate(tiles):
                    mi = 0 if t == 0 else (2 if t == T - 1 else 1)
                    if runs and runs[-1][2] == mi:
                        runs[-1][1] += 1
                    else:
                        runs.append([k, 1, mi])
                for k0, cnt, mi in runs:
                    if cnt == 1:
                        nc.vector.tensor_add(simS[:, t0 + k0:t0 + k0 + 1, :],
                                             psA[:, k0:k0 + 1, :],
                                             maskD_s[:, mi:mi + 1, :])
                    else:
                        nc.vector.tensor_add(simS[:, t0 + k0:t0 + k0 + cnt, :],
                                             psA[:, k0:k0 + cnt, :],
                                             bcast_mid(maskD_s[:, mi:mi + 1, :], cnt))
                # the shift only needs an approximate max: GpSimd makes a
                # bf16 copy so Vector's reduce runs at the 2x 16-bit rate;
                # the subtract still reads the f32 sim (full precision)
                nc.gpsimd.tensor_copy(simH[:, t0:t0 + K, :], sS)
                nc.vector.reduce_max(negmax[:, t0:t0 + K],
                                     simH[:, t0:t0 + K, :], axis=AX.X,
                                     negate=True)
                # exp arg = sim - max = sS + negmax
                nc.gpsimd.tensor_add(simB[:, t0:t0 + K, :], sS,
                                     bcast(negmax[:, t0:t0 + K], WIN))
                nc.scalar.activation(EB[:, t0:t0 + K, :], simB[:, t0:t0 + K, :],
                                     AF.Exp)

                psT = ptc.tile([128, GRP, RT], BF16, tag="ptc")
                for k, t in enumerate(tiles):
                    nc.tensor.transpose(psT[:, k, :], EB[:, t, :], ident)
                attnT = work.tile([128, GRP, RT], BF16)
                nc.scalar.copy(attnT[:, 0:K, :], psT[:, 0:K, :])

                psC = ptc.tile([RT, GRP, L + 1], F32, tag="ptc")
                for k, t in enumerate(tiles):
                    nc.tensor.matmul(psC[:, k, :], lhsT=attnT[:, k, :],
                                     rhs=memNB_s[:, t, :], start=True, stop=True)

                nc.vector.reciprocal(rinv[:, t0:t0 + K], psC[:, 0:K, L])
                for k, t in enumerate(tiles):
                    nc.vector.scalar_tensor_tensor(
                        outn[:, t, :], psC[:, k, 0:L], rinv[:, t:t + 1],
                        xh_s[:, t, :], ALU.mult, ALU.add)

                # ship this group's rows while later groups still compute;
                # alternate queues so the issues don't FIFO-serialize
                oq = nc.sync if g % 2 == 0 else nc.scalar
                oq.dma_start(
                    out=out.ap().rearrange("p (t d) -> p t d", d=L)[:, t0:t0 + K, :],
                    in_=outn[:, t0:t0 + K, :])

    nc.compile()
    return nc


def _host_prep(input, state_memory, conv_w, conv_b, bn_gamma, bn_beta, bn_mean, bn_var):
    from ml_dtypes import bfloat16

    s = (bn_gamma / np.sqrt(bn_var + BN_EPS)).astype(np.float32)
    bias_h = ((conv_b - bn_mean) * s + bn_beta).astype(np.float32)
    scb = np.ascontiguousarray(np.stack([s, bias_h], axis=1))          # [H, 2]
    wT = np.ascontiguousarray(conv_w[:, 0, :].T).astype(np.float32)    # [L, H]

    # Per-tile mask [RT, WIN]: tile t covers queries i = RT*t + r, keys
    # j = RT*t - 8 + c  (c = local col). Band |i-j| <= 8 -> c in [r, r+16],
    # clipped by 0 <= j < N and i < N.
    def tile_mask(t):
        m = np.full((RT, WIN), -1e10, dtype=np.float32)
        for r in range(RT):
            i = RT * t + r
            if i >= N:
                continue
            lo = max(i - HALF, 0) - (RT * t - HALF)
            hi = min(i + HALF, N - 1) - (RT * t - HALF)
            m[r, lo:hi + 1] = 0.0
        return m

    maskD = np.stack([tile_mask(0), tile_mask(1), tile_mask(T - 1)], axis=1)
    maskD = np.ascontiguousarray(maskD.reshape(RT, -1))                # [RT, 3*WIN]

    in_maps = []
    for b in range(B):
        x = np.ascontiguousarray(input[b, :, 0, :]).astype(np.float32)
        mem = np.ascontiguousarray(state_memory[b, :, 0, :]).astype(np.float32)
        # window-aligned mem blocks: block t = rows [RT*t-8, RT*t+120)
        mnb = np.zeros((T, 128, L + 1), dtype=np.float32)
        half_mem = (1.0 - RATE) * mem
        for t in range(T):
            lo = RT * t - HALF
            a, bnd = max(0, lo), min(N, lo + 128)
            if a < bnd:
                mnb[t, a - lo:bnd - lo, 0:L] = half_mem[a:bnd]
                mnb[t, a - lo:bnd - lo, L] = 1.0
        xh = np.zeros((T, RT, L), dtype=np.float32)
        xh.reshape(-1, L)[:N] = RATE * x
        in_maps.append({
            "xmA0": np.ascontiguousarray(np.concatenate(
                [wT, x.T[:, 0:512], mem.T[:, 0:512]], axis=1)).astype(np.float16),
            "xmA1": np.ascontiguousarray(np.concatenate(
                [x.T[:, 512:1024], mem.T[:, 512:1024]], axis=1)).astype(np.float16),
            "xmB": np.ascontiguousarray(np.concatenate(
                [x.T[:, 1024:2048], mem.T[:, 1024:2048]], axis=1)).astype(np.float16),
            "idt": np.eye(RT, dtype=np.float32).astype(bfloat16),
            "scb": scb,
            "maskD": maskD.astype(bfloat16),
            "memNB": np.ascontiguousarray(
                mnb.transpose(1, 0, 2).reshape(128, -1)).astype(bfloat16),
            "xhp": np.ascontiguousarray(
                xh.transpose(1, 0, 2).reshape(RT, -1)).astype(np.float16),
        })
    return in_maps


def run(inputs, trace=False):
    from concourse.bass_utils import run_bass_kernel_spmd
    if "nc" not in _cache:
        _cache["nc"] = _build_program()
    nc = _cache["nc"]
    in_maps = _host_prep(**inputs)
    res = run_bass_kernel_spmd(nc, in_maps, core_ids=list(range(B)), trace=trace)
    out = np.empty((B, N, C, L), dtype=np.float32)
    for b in range(B):
        o = res.results[b]["out"].astype(np.float32).reshape(RT, T, L).transpose(1, 0, 2)
        out[b] = o.reshape(NPAD, L)[:N].reshape(N, C, L)
    return out, res


def kernel(**inputs):
    out, _ = run(inputs, trace=False)
    return out


# revision 62
# speedup vs baseline: 1.2080x; 1.2080x over previous
"""Trainium2 Bass kernel for MemorySpatialAttention.

Math (per batch b):
  f = LeakyReLU_0.1(BN(conv(x)))  with conv = full-length dot -> x[N,L] @ W[L,H]
  sim = f_in @ f_mem^T  banded to |i-j| <= 8 (17 neighbors, clamped at edges)
  attn = softmax_band(sim);  out = 0.5*x + 0.5*(attn @ mem)

Sharding: data-parallel over batch B=8 -> one batch per NeuronCore, no
collectives.

Per-core structure: features in [H=128, N] layout (fp16 operands -> 1-pass
matmuls). Queries tiled 112 rows at a time with a uniform 128-wide key
window; tiles processed in groups of 4 sharing one PSUM bank so five groups
pipeline across engines. Feature activations are split into 512-wide chunks
interleaved with the group emission so group 0's QK only waits for the
first chunk pair (x/mem ship as fp16 in three DMAs, the first carrying just
the weights + first chunk so it lands early). The band mask is 3 distinct
[112,128] bf16 tiles (first/mid/last) broadcast with stride-0 APs. The
softmax chain is spread across engines: mask-add + row-max + all other
PSUM-reading ops on Vector, the max-subtract on GpSimd, exp + the psT->SBUF
copy on Scalar. The softmax denominator rides the attn@mem matmul as an
appended ones-column of memNB; the epilogue is a fused per-tile
(psC * 1/denom) + 0.5x scalar_tensor_tensor on Vector writing fp16.
Output (fp16) DMA is issued per-group on alternating Sync/Scalar queues
so transfers overlap the remaining compute and the issues don't serialize.
"""
import sys
sys.path.insert(0, '/opt/trn_rl_repo')

import numpy as np

B, N, C, L, H = 8, 2048, 1, 56, 128
NB, HALF = 17, 8
RT = 112              # query rows per tile
WIN = 128             # key window per tile
T = (N + RT - 1) // RT  # 19 tiles (last partial: 32 rows)
GRP = 4
NG = (T + GRP - 1) // GRP  # 5 groups (4,4,4,4,3 tiles)
NPAD = RT * T         # 2128
RATE = 0.5
BN_EPS = 1e-5
NEG_SLOPE = 0.1
FI_PAD = NPAD         # fiT cols (2128)
FM_PAD = HALF + N + (RT * (T - 1) + WIN - N)  # 8 + 2048 + 88 = 2144

_cache = {}


def _build_program():
    import concourse.bass as bass
    import concourse.bacc as bacc
    import concourse.tile as tile
    from concourse import mybir

    F32 = mybir.dt.float32
    F16 = mybir.dt.float16
    BF16 = mybir.dt.bfloat16
    AF = mybir.ActivationFunctionType
    AX = mybir.AxisListType
    from concourse.alu_op_type import AluOpType as ALU

    def bcast(ap_slice, n):
        return bass.AP(tensor=ap_slice.tensor, offset=ap_slice.offset,
                       ap=[*ap_slice.ap, [0, n]])

    def bcast_mid(ap_slice, k):
        # [112, 1, 128] slice -> [112, k, 128] with stride-0 middle dim
        ap = [list(d) for d in ap_slice.ap]
        assert ap[1][1] == 1, ap
        ap[1] = [0, k]
        return bass.AP(tensor=ap_slice.tensor, offset=ap_slice.offset, ap=ap)

    nc = bacc.Bacc("TRN2", target_bir_lowering=False, debug=False)

    xmA0 = nc.dram_tensor("xmA0", [L, H + 1024], F16, kind="ExternalInput")
    xmA1 = nc.dram_tensor("xmA1", [L, 1024], F16, kind="ExternalInput")
    xmB = nc.dram_tensor("xmB", [L, N], F16, kind="ExternalInput")
    scb = nc.dram_tensor("scb", [H, 2], F32, kind="ExternalInput")
    idt = nc.dram_tensor("idt", [RT, RT], BF16, kind="ExternalInput")
    maskD = nc.dram_tensor("maskD", [RT, 3 * WIN], BF16, kind="ExternalInput")
    memNB = nc.dram_tensor("memNB", [128, T * (L + 1)], BF16, kind="ExternalInput")
    xhp = nc.dram_tensor("xhp", [RT, T * L], F16, kind="ExternalInput")
    out = nc.dram_tensor("out", [RT, T * L], F16, kind="ExternalOutput")

    with tile.TileContext(nc) as tc:
        with tc.tile_pool(name="consts", bufs=1) as consts, \
             tc.tile_pool(name="work", bufs=4) as work, \
             tc.tile_pool(name="pbig", bufs=4, space="PSUM") as pbig, \
             tc.tile_pool(name="ptc", bufs=3, space="PSUM") as ptc:

            xmA0_s = consts.tile([L, H + 1024], F16)
            xmA1_s = consts.tile([L, 1024], F16)
            xmB_s = consts.tile([L, N], F16)
            wT_s = xmA0_s[:, 0:H]
            scb_s = consts.tile([H, 2], F32)
            maskD_s = consts.tile([RT, 3, WIN], BF16)
            memNB_s = consts.tile([128, T, L + 1], BF16)
            xh_s = consts.tile([RT, T, L], F16)
            ident = consts.tile([RT, RT], BF16)
            fiT = consts.tile([H, FI_PAD], F16)
            fmT = consts.tile([H, FM_PAD], F16)
            simS = consts.tile([RT, T, WIN], F32)
            simB = consts.tile([RT, T, WIN], BF16)
            EB = consts.tile([RT, T, WIN], BF16)
            negmax = consts.tile([RT, T], F32)
            rinv = consts.tile([RT, T], F32)
            outn = consts.tile([RT, T, L], F16)

            # input DMAs, first-needed first, spread over the 3 DMA queues;
            # xmA0 (weights + first x/mem chunk) goes alone first so the
            # first feature matmul isn't stuck behind the whole input set
            nc.sync.dma_start(out=xmA0_s, in_=xmA0.ap())
            nc.scalar.dma_start(out=scb_s, in_=scb.ap())
            nc.scalar.dma_start(out=xmB_s, in_=xmB.ap())
            nc.gpsimd.dma_start(out=xmA1_s, in_=xmA1.ap())
            nc.gpsimd.dma_start(out=maskD_s, in_=maskD.ap().rearrange(
                "p (t w) -> p t w", w=WIN))
            nc.gpsimd.dma_start(out=ident, in_=idt.ap())
            nc.gpsimd.dma_start(out=memNB_s, in_=memNB.ap().rearrange(
                "p (t d) -> p t d", d=L + 1))
            nc.gpsimd.dma_start(out=xh_s, in_=xhp.ap().rearrange(
                "p (t d) -> p t d", d=L))
            nc.gpsimd.memset(fmT[:, 0:HALF], 0.0)
            nc.gpsimd.memset(fmT[:, HALF + N:FM_PAD], 0.0)
            nc.gpsimd.memset(fiT[:, N:FI_PAD], 0.0)

            # ---- features in 512-col chunks: f^T = PRelu(BN(W^T @ x^T)) ----
            CH = 512

            def feat_chunk(q):  # q = 0..3 -> cols [512q, 512q+512) of x/mem
                for dst, off, m in ((fiT, 0, 0), (fmT, HALF, 1)):
                    psF = pbig.tile([128, CH], F32, tag="pbig", name="psF")
                    if q == 0:
                        src = xmA0_s[:, H + CH * m:H + CH * (m + 1)]
                    elif q == 1:
                        src = xmA1_s[:, CH * m:CH * (m + 1)]
                    else:
                        src = xmB_s[:, 1024 * m + CH * (q - 2):1024 * m + CH * (q - 1)]
                    nc.tensor.matmul(psF, lhsT=wT_s, rhs=src, start=True, stop=True)
                    nc.scalar.activation(dst[:, off + CH * q:off + CH * (q + 1)],
                                         psF, AF.Prelu, bias=scb_s[:, 1:2],
                                         scale=scb_s[:, 0:1], alpha=NEG_SLOPE)

            # group g needs feature cols up to ~448(g+1)+8: chunk (0,1,2,3)
            # ready before groups (0, 1, 2, 3) respectively.
            feat_chunk(0)
            feat_chunk(1)

            # ---- banded attention in groups of GRP tiles ----
            for g in range(NG):
                if g >= 1 and g + 1 < 4:
                    feat_chunk(g + 1)

                tiles = list(range(g * GRP, min((g + 1) * GRP, T)))
                K = len(tiles)
                t0 = tiles[0]

                psA = pbig.tile([RT, GRP, WIN], F32, tag="pbig", name="psA")
                for k, t in enumerate(tiles):
                    nc.tensor.matmul(psA[:, k, :], lhsT=fiT[:, RT * t:RT * (t + 1)],
                                     rhs=fmT[:, RT * t:RT * t + WIN],
                                     start=True, stop=True)

                # sS = psA + mask; one add per run of equal mask tile
                sS = simS[:, t0:t0 + K, :]
                runs = []  # (k_start, count, mi)
                for k, t in enumerate(tiles):
                    mi = 0 if t == 0 else (2 if t == T - 1 else 1)
                    if runs and runs[-1][2] == mi:
                        runs[-1][1] += 1
                    else:
                        runs.append([k, 1, mi])
                for k0, cnt, mi in runs:
                    if cnt == 1:
                        nc.vector.tensor_add(simS[:, t0 + k0:t0 + k0 + 1, :],
                                             psA[:, k0:k0 + 1, :],
                                             maskD_s[:, mi:mi + 1, :])
                    else:
                        nc.vector.tensor_add(simS[:, t0 + k0:t0 + k0 + cnt, :],
                                             psA[:, k0:k0 + cnt, :],
                                             bcast_mid(maskD_s[:, mi:mi + 1, :], cnt))
                nc.vector.reduce_max(negmax[:, t0:t0 + K], sS, axis=AX.X,
                                     negate=True)
                # exp arg = sim - max = sS + negmax
                nc.gpsimd.tensor_add(simB[:, t0:t0 + K, :], sS,
                                     bcast(negmax[:, t0:t0 + K], WIN))
                nc.scalar.activation(EB[:, t0:t0 + K, :], simB[:, t0:t0 + K, :],
                                     AF.Exp)

                psT = ptc.tile([128, GRP, RT], BF16, tag="ptc")
                for k, t in enumerate(tiles):
                    nc.tensor.transpose(psT[:, k, :], EB[:, t, :], ident)
                attnT = work.tile([128, GRP, RT], BF16)
                nc.scalar.copy(attnT[:, 0:K, :], psT[:, 0:K, :])

                psC = ptc.tile([RT, GRP, L + 1], F32, tag="ptc")
                for k, t in enumerate(tiles):
                    nc.tensor.matmul(psC[:, k, :], lhsT=attnT[:, k, :],
                                     rhs=memNB_s[:, t, :], start=True, stop=True)

                nc.vector.reciprocal(rinv[:, t0:t0 + K], psC[:, 0:K, L])
                for k, t in enumerate(tiles):
                    nc.vector.scalar_tensor_tensor(
                        outn[:, t, :], psC[:, k, 0:L], rinv[:, t:t + 1],
                        xh_s[:, t, :], ALU.mult, ALU.add)

                # ship this group's rows while later groups still compute;
                # alternate queues so the issues don't FIFO-serialize
                oq = nc.sync if g % 2 == 0 else nc.scalar
                oq.dma_start(
                    out=out.ap().rearrange("p (t d) -> p t d", d=L)[:, t0:t0 + K, :],
                    in_=outn[:, t0:t0 + K, :])

    nc.compile()
    return nc


def _host_prep(input, state_memory, conv_w, conv_b, bn_gamma, bn_beta, bn_mean, bn_var):
    from ml_dtypes import bfloat16

    s = (bn_gamma / np.sqrt(bn_var + BN_EPS)).astype(np.float32)
    bias_h = ((conv_b - bn_mean) * s + bn_beta).astype(np.float32)
    scb = np.ascontiguousarray(np.stack([s, bias_h], axis=1))          # [H, 2]
    wT = np.ascontiguousarray(conv_w[:, 0, :].T).astype(np.float32)    # [L, H]

    # Per-tile mask [RT, WIN]: tile t covers queries i = RT*t + r, keys
    # j = RT*t - 8 + c  (c = local col). Band |i-j| <= 8 -> c in [r, r+16],
    # clipped by 0 <= j < N and i < N.
    def tile_mask(t):
        m = np.full((RT, WIN), -1e10, dtype=np.float32)
        for r in range(RT):
            i = RT * t + r
            if i >= N:
                continue
            lo = max(i - HALF, 0) - (RT * t - HALF)
            hi = min(i + HALF, N - 1) - (RT * t - HALF)
            m[r, lo:hi + 1] = 0.0
        return m

    maskD = np.stack([tile_mask(0), tile_mask(1), tile_mask(T - 1)], axis=1)
    maskD = np.ascontiguousarray(maskD.reshape(RT, -1))                # [RT, 3*WIN]

    in_maps = []
    for b in range(B):
        x = np.ascontiguousarray(input[b, :, 0, :]).astype(np.float32)
        mem = np.ascontiguousarray(state_memory[b, :, 0, :]).astype(np.float32)
        # window-aligned mem blocks: block t = rows [RT*t-8, RT*t+120)
        mnb = np.zeros((T, 128, L + 1), dtype=np.float32)
        half_mem = (1.0 - RATE) * mem
        for t in range(T):
            lo = RT * t - HALF
            a, bnd = max(0, lo), min(N, lo + 128)
            if a < bnd:
                mnb[t, a - lo:bnd - lo, 0:L] = half_mem[a:bnd]
                mnb[t, a - lo:bnd - lo, L] = 1.0
        xh = np.zeros((T, RT, L), dtype=np.float32)
        xh.reshape(-1, L)[:N] = RATE * x
        in_maps.append({
            "xmA0": np.ascontiguousarray(np.concatenate(
                [wT, x.T[:, 0:512], mem.T[:, 0:512]], axis=1)).astype(np.float16),
            "xmA1": np.ascontiguousarray(np.concatenate(
                [x.T[:, 512:1024], mem.T[:, 512:1024]], axis=1)).astype(np.float16),
            "xmB": np.ascontiguousarray(np.concatenate(
                [x.T[:, 1024:2048], mem.T[:, 1024:2048]], axis=1)).astype(np.float16),
            "idt": np.eye(RT, dtype=np.float32).astype(bfloat16),
            "scb": scb,
            "maskD": maskD.astype(bfloat16),
            "memNB": np.ascontiguousarray(
                mnb.transpose(1, 0, 2).reshape(128, -1)).astype(bfloat16),
            "xhp": np.ascontiguousarray(
                xh.transpose(1, 0, 2).reshape(RT, -1)).astype(np.float16),
        })
    return in_maps


def run(inputs, trace=False):
    from concourse.bass_utils import run_bass_kernel_spmd
    if "nc" not in _cache:
        _cache["nc"] = _build_program()
    nc = _cache["nc"]
    in_maps = _host_prep(**inputs)
    res = run_bass_kernel_spmd(nc, in_maps, core_ids=list(range(B)), trace=trace)
    out = np.empty((B, N, C, L), dtype=np.float32)
    for b in range(B):
        o = res.results[b]["out"].astype(np.float32).reshape(RT, T, L).transpose(1, 0, 2)
        out[b] = o.reshape(NPAD, L)[:N].reshape(N, C, L)
    return out, res


def kernel(**inputs):
    out, _ = run(inputs, trace=False)
    return out
